# revision 17
# baseline (speedup 1.0000x reference)
"""Trainium2 Bass kernel for nn_Aggregation (SAN-style local aggregation).

out[n, g*32+cc, h, w] = sum_{kh,kw} input[n, g*32+cc, h-3+kh, w-3+kw] * weight[n, cc, kh*7+kw, h, w]

Sharding: data-parallel over batch N=16 across 8 NeuronCores (2 images/core).

Per-core layout:
  partition p = cc*4 + blk   (cc in [0,32): weight channel, blk in [0,4): block of 8 output rows)
  in_pad[p][n, g, r, col] = zero-padded input rows [blk*8-3, blk*8+11), cols [-3, 35)
  w_t[p][n, kk, hb, w]    = weight[n, cc, kk, blk*8+hb, w]
  For each tap kk=(kh,kw): acc[p][n,g,hb,w] += in_pad[p][n,g,hb+kh,w+kw] * w_t[p][n,kk,hb,w]
  (weight broadcast over g via stride-0 access pattern)

Mode "fp16row" (default): products and within-row (7-tap) accumulation in fp16
on the DVE at 2x rate; row sums flushed into an fp32 accumulator. A second
input copy shifted by one column keeps odd-kw taps 4B-aligned so the DVE's
2x perf mode stays engaged. Max abs error vs fp32 reference ~7e-4 of absmax.
Mode "fp32": everything fp32 (exact, ~2x slower).
"""

import numpy as np

N, C, H, W = 16, 256, 32, 32
K, PAD = 7, 3
CC, G = 32, 8
KK = K * K
NCORES = 8
NPC = N // NCORES
BLK, HB = 4, 8
R, COLP = HB + 2 * PAD, W + 2 * PAD  # 14, 38

MODE = "v2"

_cache = {}


def _build(mode):
    import concourse.bacc as bacc
    import concourse.mybir as mybir
    import concourse.tile as tile

    fp32 = mybir.dt.float32
    fp16 = mybir.dt.float16
    cdt = fp32 if mode == "fp32" else fp16  # compute dtype
    mult = mybir.AluOpType.mult
    add = mybir.AluOpType.add

    nc = bacc.Bacc("TRN2", target_bir_lowering=False, debug=False, num_devices=NCORES)
    x = nc.dram_tensor("input", [NPC, C, H, W], fp32, kind="ExternalInput").ap()
    wgt = nc.dram_tensor("weight", [NPC, CC, KK, H, W], fp32, kind="ExternalInput").ap()
    y = nc.dram_tensor("output", [NPC, C, H, W], fp32, kind="ExternalOutput").ap()
    if mode in ("fp16pe", "v2"):
        idn = nc.dram_tensor("identity", [128, 128], fp16, kind="ExternalInput").ap()

    if mode == "v2":
        _build_v2(nc, x, wgt, y, idn)
        nc.compile()
        return nc

    with tile.TileContext(nc) as tc:
        with (
            tc.tile_pool(name="main", bufs=1) as pool,
            tc.tile_pool(name="prod", bufs=4) as ppool,
            tc.tile_pool(name="rowp", bufs=3) as rpool,
            tc.tile_pool(name="tree", bufs=1) as tpool,
            tc.tile_pool(name="psum", bufs=1, space="PSUM") as pspool,
        ):
            in_pad = pool.tile([128, NPC, G, R, COLP], cdt)
            stage = pool.tile([128, NPC, G, R, W], fp32)
            w_t = pool.tile([128, NPC, KK, HB, W], cdt)
            acc = pool.tile([128, NPC, G, HB, W], fp32)
            if mode != "fp32":
                in_pad1 = pool.tile([128, NPC, G, R, COLP], cdt)
            if mode == "fp16pe":
                ident = pool.tile([128, 128], cdt)
                nc.sync.dma_start(out=ident[:], in_=idn[:])
                acc_ps = pspool.tile([128, NPC * G * HB * W], fp32)
                # touch ACT once so its table set loads during the DMA wait
                warm = pool.tile([128, 1], fp32)
                nc.scalar.copy(out=warm[:], in_=ident[:, 0:1])

            # Zero only the halo regions: left/right column borders of
            # in_pad/in_pad1, and the top/bottom staging row-bands that flow
            # into the padded rows. On the DVE: it is idle at kernel start and
            # clears these fast, unblocking the staging DMAs (WAW).
            # full-partition stage row bands first: they gate the staging DMAs
            # (the DMAs overwrite whichever rows are valid for their block)
            nc.vector.memset(stage[:, :, :, 0:PAD, :], 0.0)
            nc.vector.memset(stage[:, :, :, R - PAD : R, :], 0.0)
            nc.vector.memset(in_pad[:, :, :, :, 0:PAD], 0.0)
            nc.vector.memset(in_pad[:, :, :, :, PAD + W : COLP], 0.0)
            if mode != "fp32":
                # in_pad1 holds input shifted one column left: interior
                # cols [PAD-1, PAD-1+W), borders outside that
                nc.vector.memset(in_pad1[:, :, :, :, 0 : PAD - 1], 0.0)
                nc.vector.memset(in_pad1[:, :, :, :, PAD - 1 + W : COLP], 0.0)

            # Input staging loads on HWDGE (fp32), then two independent
            # convert-copies of stage: ACT builds in_pad (interior at col PAD),
            # DVE builds in_pad1 (interior at col PAD-1, i.e. shifted one
            # column left, which keeps odd-kw taps 4B-aligned for the DVE 2x
            # perf mode). Weights stream via SWDGE dtype-cast DMAs in two
            # kh-chunks so compute can start after chunk A lands.
            stage_dmas = []
            # two row-slabs per (n, blk): slab 1 covers padded rows [0, 8)
            # (everything the kh=0 taps read), slab 2 the rest, so compute can
            # start as soon as slab 1 lands
            SLABS = [(0, HB), (HB, R)] if mode != "fp32" else [(0, R)]
            for ra, rb in SLABS:
                for n in range(NPC):
                    for blk in range(BLK):
                        # valid input rows within padded window rows [ra, rb)
                        h0 = max(0, blk * HB - PAD + ra)
                        h1 = min(H, blk * HB - PAD + rb)
                        if h1 <= h0:
                            continue
                        r0 = h0 - (blk * HB - PAD)
                        dst = stage[blk::BLK, n].rearrange("p g r c -> p g (r c)")[
                            :, :, r0 * W : (r0 + (h1 - h0)) * W
                        ]
                        src = x[n].rearrange("(g cc) h w -> cc g (h w)", g=G)[
                            :, :, h0 * W : h1 * W
                        ]
                        ieng = nc.sync if blk % 2 == 0 else nc.scalar
                        sd = ieng.dma_start(out=dst, in_=src)
                        stage_dmas.append(sd)
            # Convert-copies split across ACT and DVE (DVE single-src fp32
            # copies run at 2x): ACT pads image 0; DVE shifts image 0, then
            # pads+shifts image 1.
            if mode == "fp32":
                for n in range(NPC):
                    nc.scalar.copy(
                        out=in_pad[:, n, :, :, PAD : PAD + W], in_=stage[:, n]
                    )
            else:
                # slab 1 (rows 0:8, the kh=0 critical path): one copy on ACT,
                # three on DVE (2x single-src fp32 copies). Slab 2 goes
                # entirely to ACT, which is idle while kh=0 computes.
                ra, rb = SLABS[0]
                nc.scalar.copy(
                    out=in_pad[:, 0, :, ra:rb, PAD : PAD + W],
                    in_=stage[:, 0, :, ra:rb],
                )
                nc.vector.tensor_copy(
                    out=in_pad1[:, 0, :, ra:rb, PAD - 1 : PAD - 1 + W],
                    in_=stage[:, 0, :, ra:rb],
                )
                nc.vector.tensor_copy(
                    out=in_pad[:, 1, :, ra:rb, PAD : PAD + W],
                    in_=stage[:, 1, :, ra:rb],
                )
                nc.vector.tensor_copy(
                    out=in_pad1[:, 1, :, ra:rb, PAD - 1 : PAD - 1 + W],
                    in_=stage[:, 1, :, ra:rb],
                )
                ra, rb = SLABS[1]
                for n in range(NPC):
                    nc.scalar.copy(
                        out=in_pad[:, n, :, ra:rb, PAD : PAD + W],
                        in_=stage[:, n, :, ra:rb],
                    )
                    nc.scalar.copy(
                        out=in_pad1[:, n, :, ra:rb, PAD - 1 : PAD - 1 + W],
                        in_=stage[:, n, :, ra:rb],
                    )
            # Weight chunks: A loads immediately; later chunks are held back
            # (dep on an early multiply) so their HBM traffic does not starve
            # the input staging path at startup. Deps are attached after the
            # compute loop records its mult instructions.
            KH_CHUNKS = [(0, 1), (1, 2), (2, 4), (4, K)] if mode != "fp32" else [(0, K)]
            CHUNK_DEP_KK = [None, 0, 2, 14]
            wdmas = []  # (chunk_idx, BassInstruction)
            for ci, (kh0, kh1) in enumerate(KH_CHUNKS):
                for n in range(NPC):
                    wsrc = wgt[n].rearrange(
                        "cc kk (blk hb) w -> cc blk kk (hb w)", blk=BLK
                    )
                    for blk in range(BLK):
                        d = nc.gpsimd.dma_start(
                            out=w_t[blk::BLK, n, kh0 * K : kh1 * K].rearrange(
                                "p kk hb w -> p kk (hb w)"
                            ),
                            in_=wsrc[:, blk, kh0 * K : kh1 * K],
                        )
                        wdmas.append((ci, d))

            if mode == "fp32":
                for n in range(NPC):
                    for kh in range(K):
                        for kw in range(K):
                            kk = kh * K + kw
                            in0 = in_pad[:, n, :, kh : kh + HB, kw : kw + W]
                            in1 = (
                                w_t[:, n, kk].unsqueeze(1).broadcast_to([128, G, HB, W])
                            )
                            if kk == 0:
                                nc.vector.tensor_tensor(
                                    out=acc[:, n], in0=in0, in1=in1, op=mult
                                )
                            else:
                                prod = ppool.tile([128, G, HB, W], cdt)
                                nc.vector.tensor_tensor(
                                    out=prod[:], in0=in0, in1=in1, op=mult
                                )
                                nc.vector.tensor_tensor(
                                    out=acc[:, n], in0=acc[:, n], in1=prod[:], op=add
                                )
            elif mode == "fp16pe":
                # DVE computes fp16 products at its 2x rate; the otherwise
                # idle Tensor engine accumulates them into an fp32 PSUM
                # accumulator via identity matmuls (start clears, subsequent
                # matmuls accumulate via PSUM has_written bits). Accumulation
                # is therefore exact fp32: only products are rounded to fp16.
                shp = [128, NPC, G, HB, W]
                FLAT = NPC * G * HB * W  # 4096 fp32 = exactly 8 PSUM banks
                NBANK = FLAT // 512
                mults = []
                for kk in range(KK):
                    kh, kw = divmod(kk, K)
                    if kw % 2 == 0:
                        in0 = in_pad[:, :, :, kh : kh + HB, kw : kw + W]
                    else:
                        in0 = in_pad1[:, :, :, kh : kh + HB, kw - 1 : kw - 1 + W]
                    in1 = w_t[:, :, kk].unsqueeze(2).broadcast_to(shp)
                    prod = ppool.tile(shp, cdt)
                    m = nc.vector.tensor_tensor(
                        out=prod[:], in0=in0, in1=in1, op=mult
                    )
                    mults.append(m)
                    pf = prod[:].rearrange("p n g h w -> p (n g h w)")
                    for b in range(NBANK):
                        nc.tensor.matmul(
                            out=acc_ps[:, 512 * b : 512 * (b + 1)],
                            lhsT=ident[:],
                            rhs=pf[:, 512 * b : 512 * (b + 1)],
                            start=(kk == 0),
                            stop=(kk == KK - 1),
                        )
                # evict PSUM -> SBUF (fp32) in quarters, alternating DVE (fast
                # 2x fp32 copy, idle once the multiplies are done) and ACT, so
                # stores can start while later quarters are still copying
                av = acc[:].rearrange("p n g h w -> p (n g h w)")
                q = FLAT // 4
                for i in range(4):
                    eng = nc.vector.tensor_copy if i % 2 == 0 else nc.scalar.copy
                    eng(out=av[:, i * q : (i + 1) * q], in_=acc_ps[:, i * q : (i + 1) * q])
                from concourse.bass import _add_dep_helper

                for ci, d in wdmas:
                    dep_kk = CHUNK_DEP_KK[ci]
                    if dep_kk == "stage":
                        _add_dep_helper(
                            d.ins,
                            stage_dmas[-1].ins,
                            sync=True,
                            reason="weight chunk A transfers after input staging",
                        )
                    elif dep_kk is not None:
                        _add_dep_helper(
                            d.ins,
                            mults[dep_kk].ins,
                            sync=True,
                            reason=f"delay weight chunk {ci} past startup",
                        )
            else:
                # both images processed by each instruction (n on a free axis)
                shp = [128, NPC, G, HB, W]
                prev_row = None
                treeacc = tpool.tile(shp, cdt, tag="treeacc")
                for kh in range(K):
                    rowacc = rpool.tile(shp, cdt)
                    for kw in range(K):
                        kk = kh * K + kw
                        if kw % 2 == 0:
                            in0 = in_pad[:, :, :, kh : kh + HB, kw : kw + W]
                        else:
                            in0 = in_pad1[:, :, :, kh : kh + HB, kw - 1 : kw - 1 + W]
                        in1 = w_t[:, :, kk].unsqueeze(2).broadcast_to(shp)
                        if kw == 0:
                            nc.vector.tensor_tensor(
                                out=rowacc[:], in0=in0, in1=in1, op=mult
                            )
                        else:
                            prod = ppool.tile(shp, cdt)
                            nc.vector.tensor_tensor(
                                out=prod[:], in0=in0, in1=in1, op=mult
                            )
                            nc.vector.tensor_tensor(
                                out=rowacc[:], in0=rowacc[:], in1=prod[:], op=add
                            )
                    # pairwise fp16 combine of row sums (keeps the fp16 2x
                    # rate; only the very last add writes fp32)
                    if kh == K - 1:
                        # split by g-half so the first half's output stores can
                        # overlap the second half's add
                        half = G // 2
                        nc.vector.tensor_tensor(
                            out=acc[:, :, 0:half],
                            in0=treeacc[:, :, 0:half],
                            in1=rowacc[:, :, 0:half],
                            op=add,
                        )
                        nc.vector.tensor_tensor(
                            out=acc[:, :, half:G],
                            in0=treeacc[:, :, half:G],
                            in1=rowacc[:, :, half:G],
                            op=add,
                        )
                    elif kh % 2 == 1:
                        if kh == 1:
                            nc.vector.tensor_tensor(
                                out=treeacc[:], in0=prev_row[:], in1=rowacc[:], op=add
                            )
                        else:
                            pair = tpool.tile(shp, cdt, tag="pair")
                            nc.vector.tensor_tensor(
                                out=pair[:], in0=prev_row[:], in1=rowacc[:], op=add
                            )
                            nc.vector.tensor_tensor(
                                out=treeacc[:], in0=treeacc[:], in1=pair[:], op=add
                            )
                        prev_row = None
                    else:
                        prev_row = rowacc
            for g in range(G):
                for n in range(NPC):
                    dsty = y[n].rearrange(
                        "(g cc) (blk hb) w -> g cc blk (hb w)", g=G, blk=BLK
                    )
                    deng = nc.sync if g % 2 == 0 else nc.scalar
                    deng.dma_start(
                        out=dsty[g],
                        in_=acc[:, n, g].rearrange("p hb w -> p (hb w)"),
                    )

    nc.compile()
    return nc


def _build_v2(nc, x, wgt, y, idn):
    """Three-engine tap stream.

    Free-dim layout (g, n, hb, w) so PSUM bank b holds group g=b. Per tap:
    DVE multiplies groups 0..6 (fp16 2x), GPSIMD multiplies group 7 (software,
    alignment-insensitive so it reads in_pad directly even for odd kw), PE
    accumulates all 8 banks via identity matmuls. kh=0 taps are split by image
    n (even kw first) so compute starts as soon as image 0's rows [0,8) are
    staged and converted. Weights stream as 8 consolidated SWDGE cast DMAs
    whose generation is self-throttled by gpsimd program order. Eviction
    alternates DVE/ACT per bank with per-bank stores chasing.
    """
    import concourse.mybir as mybir
    import concourse.tile as tile

    fp32 = mybir.dt.float32
    cdt = mybir.dt.float16
    mult = mybir.AluOpType.mult
    GD = G - 1  # groups handled by DVE

    with tile.TileContext(nc) as tc:
        with (
            tc.tile_pool(name="main", bufs=1) as pool,
            tc.tile_pool(name="prod", bufs=6) as ppool,
            tc.tile_pool(name="prod7", bufs=14) as p7pool,
            tc.tile_pool(name="psum", bufs=1, space="PSUM") as pspool,
        ):
            in_pad = pool.tile([128, G, NPC, R, COLP], cdt)
            in_pad1 = pool.tile([128, G, NPC, R, COLP], cdt)
            stage = pool.tile([128, NPC, G, R, W], fp32)
            w_t = pool.tile([128, NPC, KK, HB, W], cdt)
            acc = pool.tile([128, G, NPC, HB, W], fp32)
            ident = pool.tile([128, 128], cdt)
            acc_ps = pspool.tile([128, G * NPC * HB * W], fp32)  # 8 banks

            # --- weight cast DMAs, all generated up-front on gpsimd ---
            # (SWDGE is the only caster). Rows kh0 and kh1 go as their own
            # small chunks so early taps aren't gated on the bulk; gpsimd's
            # g7 multiplies tolerate the generation time via the delayed
            # bank-7 emission below.
            wre = [
                wgt[n].rearrange("cc kk (blk hb) w -> cc blk kk (hb w)", blk=BLK)
                for n in range(NPC)
            ]

            def wchunk(kh0, kh1):
                for n in range(NPC):
                    for b in range(BLK):
                        nc.gpsimd.dma_start(
                            out=w_t[b::BLK, n, kh0 * K : kh1 * K].rearrange(
                                "p kk hb w -> p kk (hb w)"
                            ),
                            in_=wre[n][:, b, kh0 * K : kh1 * K],
                        )

            wchunk(0, 1)

            # --- halo memsets on DVE (idle until first mult) ---
            nc.vector.memset(stage[:, :, :, 0:PAD, :], 0.0)
            nc.vector.memset(stage[:, :, :, R - PAD : R, :], 0.0)
            nc.vector.memset(in_pad[:, :, :, :, 0:PAD], 0.0)
            nc.vector.memset(in_pad[:, :, :, :, PAD + W : COLP], 0.0)
            nc.vector.memset(in_pad1[:, :, :, :, 0 : PAD - 1], 0.0)
            nc.vector.memset(in_pad1[:, :, :, :, PAD - 1 + W : COLP], 0.0)
            zs = pool.tile([128, NPC * HB * W], cdt)
            nc.vector.memset(zs[:], 0.0)

            # --- input staging: big DMA per (slab, n) + 3 halo DMAs each ---
            # big: padded rows [3,8) then [8,11) are valid for every blk
            # (h = blk*8 + (r-PAD)); halo rows need per-blk clamping.
            nc.sync.dma_start(out=ident[:], in_=idn[:])
            xbig = [
                x[n].rearrange("(g cc) (blk hb) w -> (cc blk) g hb w", g=G, blk=BLK)
                for n in range(NPC)
            ]
            xh = [
                x[n].rearrange("(g cc) h w -> cc g h w", g=G) for n in range(NPC)
            ]
            for n in range(NPC):
                nc.sync.dma_start(
                    out=stage[:, n, :, PAD : PAD + 5, :], in_=xbig[n][:, :, 0:5, :]
                )
            for n in range(NPC):
                for b in (1, 2, 3):  # top halo rows [0,3): h in [b*8-3, b*8)
                    nc.scalar.dma_start(
                        out=stage[b::BLK, n, :, 0:PAD, :],
                        in_=xh[n][:, :, b * HB - PAD : b * HB, :],
                    )
            for n in range(NPC):
                nc.sync.dma_start(
                    out=stage[:, n, :, PAD + 5 : PAD + HB, :],
                    in_=xbig[n][:, :, 5:HB, :],
                )
            for n in range(NPC):
                for b in (0, 1, 2):  # bottom halo rows [11,14): h in [b*8+8, b*8+11)
                    nc.scalar.dma_start(
                        out=stage[b::BLK, n, :, R - PAD : R, :],
                        in_=xh[n][:, :, b * HB + HB : b * HB + HB + PAD, :],
                    )

            # touch ACT once so its table set loads during the DMA wait
            warm = pool.tile([128, 1], fp32)
            nc.scalar.copy(out=warm[:], in_=ident[:, 0:1])

            # --- converts, all on ACT, ordered for earliest compute ---
            # slab1 (rows 0:8) n0 pad, n1 pad, n0 pad1, n1 pad1; then slab2.
            def conv(n, ra, rb, shifted):
                dst = in_pad1 if shifted else in_pad
                c0 = PAD - 1 if shifted else PAD
                nc.scalar.copy(
                    out=dst[:, :, n, ra:rb, c0 : c0 + W], in_=stage[:, n, :, ra:rb]
                )

            conv(0, 0, HB, False)
            conv(1, 0, HB, False)
            conv(0, 0, HB, True)
            conv(1, 0, HB, True)
            conv(0, HB, R, False)
            conv(1, HB, R, False)
            conv(0, HB, R, True)
            conv(1, HB, R, True)

            # --- PSUM init: zero-matmuls open every bank's accumulation
            # group (HW start_tensor_calc is bank-wide, so per-image starts
            # would wipe earlier partial sums). Also warms the PE pipeline
            # during the staging wait. All tap matmuls accumulate.
            shp_j = [128, GD, NPC, HB, W]
            SPAN = NPC * HB * W  # 512 elems = one PSUM bank
            DELAY7 = 12
            for g in range(G):
                nc.tensor.matmul(
                    out=acc_ps[:, SPAN * g : SPAN * (g + 1)],
                    lhsT=ident[:],
                    rhs=zs[:],
                    start=True,
                    stop=False,
                )

            def dve_mult(kk, prod_t):
                kh, kw = divmod(kk, K)
                src, c0 = (in_pad, kw) if kw % 2 == 0 else (in_pad1, kw - 1)
                in0 = src[:, 0:GD, :, kh : kh + HB, c0 : c0 + W]
                in1 = w_t[:, :, kk].unsqueeze(1).broadcast_to(shp_j)
                return nc.vector.tensor_tensor(
                    out=prod_t[:], in0=in0, in1=in1, op=mult
                )

            def gp_mult(kk, p7_t):
                kh, kw = divmod(kk, K)
                src, c0 = (in_pad, kw) if kw % 2 == 0 else (in_pad1, kw - 1)
                in0 = src[:, GD, :, kh : kh + HB, c0 : c0 + W]
                in1 = w_t[:, :, kk]
                return nc.gpsimd.tensor_tensor(
                    out=p7_t[:], in0=in0, in1=in1, op=mult
                )

            def main_matmuls(prod_t, last=False):
                pf = prod_t[:].rearrange("p g n h w -> p (g n h w)")
                for g in range(GD):
                    nc.tensor.matmul(
                        out=acc_ps[:, SPAN * g : SPAN * (g + 1)],
                        lhsT=ident[:],
                        rhs=pf[:, SPAN * g : SPAN * (g + 1)],
                        start=False,
                        stop=last,
                    )

            pending7 = []

            def bank7_matmul(p7_t, last=False):
                nc.tensor.matmul(
                    out=acc_ps[:, SPAN * GD : SPAN * G],
                    lhsT=ident[:],
                    rhs=p7_t[:].rearrange("p n h w -> p (n h w)"),
                    start=False,
                    stop=last,
                )

            def drain7(limit):
                while len(pending7) > limit:
                    bank7_matmul(*pending7.pop(0))

            # joint taps; within kh=0 run even kw first so odd taps wait on
            # in_pad1 converts without stalling the stream
            ORDER = [0, 2, 4, 6, 1, 3, 5] + list(range(K, KK))
            for i, kk in enumerate(ORDER):
                prod_t = ppool.tile([128, GD, NPC, HB, W], cdt)
                p7_t = p7pool.tile([128, NPC, HB, W], cdt)
                dve_mult(kk, prod_t)
                gp_mult(kk, p7_t)
                main_matmuls(prod_t, last=(kk == KK - 1))
                pending7.append([p7_t, kk == KK - 1])
                # shrink the bank-7 backlog toward the end so the PE queue
                # finishes almost immediately after the last main matmul
                drain7(min(DELAY7, len(ORDER) - 1 - i))
                # queue the next weight-row chunk a few taps ahead of its
                # kh row; gpsimd program order self-throttles the generation
                if i == 3:
                    wchunk(1, 2)
                elif i == 10:
                    wchunk(2, 3)
                elif i == 17:
                    wchunk(3, 5)
                elif i == 27:
                    wchunk(5, K)
            drain7(0)

            # --- eviction + stores, per bank, alternating engines/queues ---
            yr = y.rearrange("n (g cc) (blk hb) w -> g (cc blk) n (hb w)", g=G, blk=BLK)
            span = NPC * HB * W
            for g in range(G):
                ev = nc.vector.tensor_copy if g % 2 == 0 else nc.scalar.copy
                ev(
                    out=acc[:, g].rearrange("p n h w -> p (n h w)"),
                    in_=acc_ps[:, span * g : span * (g + 1)],
                )
                deng = nc.sync if g % 2 == 0 else nc.scalar
                deng.dma_start(
                    out=yr[g], in_=acc[:, g].rearrange("p n h w -> p n (h w)")
                )


def _get_nc(mode=None):
    mode = mode or MODE
    if mode not in _cache:
        _cache[mode] = _build(mode)
    return _cache[mode]


def kernel(input_, weight, _trace=False, _mode=None):
    from concourse.bass_utils import run_bass_kernel_spmd

    mode = _mode or MODE
    nc = _get_nc(mode)
    input_ = np.ascontiguousarray(input_, dtype=np.float32)
    weight = np.ascontiguousarray(weight, dtype=np.float32)
    eye = np.eye(128, dtype=np.float16)
    in_maps = [
        {
            "input": input_[i * NPC : (i + 1) * NPC],
            "weight": weight[i * NPC : (i + 1) * NPC],
            **({"identity": eye} if mode in ("fp16pe", "v2") else {}),
        }
        for i in range(NCORES)
    ]
    res = run_bass_kernel_spmd(nc, in_maps, list(range(NCORES)), trace=_trace)
    _cache["last_result"] = res
    out = np.concatenate([res.results[i]["output"] for i in range(NCORES)], axis=0)
    return out



# revision 18
# speedup vs baseline: 1.1226x; 1.1226x over previous
"""Trainium2 Bass kernel for nn_Aggregation (SAN-style local aggregation).

out[n, g*32+cc, h, w] = sum_{kh,kw} input[n, g*32+cc, h-3+kh, w-3+kw] * weight[n, cc, kh*7+kw, h, w]

Sharding: data-parallel over batch N=16 across 8 NeuronCores (2 images/core).

Per-core layout:
  partition p = cc*4 + blk   (cc in [0,32): weight channel, blk in [0,4): block of 8 output rows)
  in_pad[p][n, g, r, col] = zero-padded input rows [blk*8-3, blk*8+11), cols [-3, 35)
  w_t[p][n, kk, hb, w]    = weight[n, cc, kk, blk*8+hb, w]
  For each tap kk=(kh,kw): acc[p][n,g,hb,w] += in_pad[p][n,g,hb+kh,w+kw] * w_t[p][n,kk,hb,w]
  (weight broadcast over g via stride-0 access pattern)

Mode "fp16row" (default): products and within-row (7-tap) accumulation in fp16
on the DVE at 2x rate; row sums flushed into an fp32 accumulator. A second
input copy shifted by one column keeps odd-kw taps 4B-aligned so the DVE's
2x perf mode stays engaged. Max abs error vs fp32 reference ~7e-4 of absmax.
Mode "fp32": everything fp32 (exact, ~2x slower).
"""

import numpy as np

N, C, H, W = 16, 256, 32, 32
K, PAD = 7, 3
CC, G = 32, 8
KK = K * K
NCORES = 8
NPC = N // NCORES
BLK, HB = 4, 8
R, COLP = HB + 2 * PAD, W + 2 * PAD  # 14, 38

MODE = "v2"

_cache = {}


def _build(mode):
    import concourse.bacc as bacc
    import concourse.mybir as mybir
    import concourse.tile as tile

    fp32 = mybir.dt.float32
    fp16 = mybir.dt.float16
    cdt = fp32 if mode == "fp32" else fp16  # compute dtype
    mult = mybir.AluOpType.mult
    add = mybir.AluOpType.add

    nc = bacc.Bacc("TRN2", target_bir_lowering=False, debug=False, num_devices=NCORES)
    x = nc.dram_tensor("input", [NPC, C, H, W], fp32, kind="ExternalInput").ap()
    wgt = nc.dram_tensor("weight", [NPC, CC, KK, H, W], fp32, kind="ExternalInput").ap()
    y = nc.dram_tensor("output", [NPC, C, H, W], fp32, kind="ExternalOutput").ap()
    if mode in ("fp16pe", "v2"):
        idn = nc.dram_tensor("identity", [128, 128], fp16, kind="ExternalInput").ap()

    if mode == "v2":
        _build_v2(nc, x, wgt, y, idn)
        nc.compile()
        return nc

    with tile.TileContext(nc) as tc:
        with (
            tc.tile_pool(name="main", bufs=1) as pool,
            tc.tile_pool(name="prod", bufs=4) as ppool,
            tc.tile_pool(name="rowp", bufs=3) as rpool,
            tc.tile_pool(name="tree", bufs=1) as tpool,
            tc.tile_pool(name="psum", bufs=1, space="PSUM") as pspool,
        ):
            in_pad = pool.tile([128, NPC, G, R, COLP], cdt)
            stage = pool.tile([128, NPC, G, R, W], fp32)
            w_t = pool.tile([128, NPC, KK, HB, W], cdt)
            acc = pool.tile([128, NPC, G, HB, W], fp32)
            if mode != "fp32":
                in_pad1 = pool.tile([128, NPC, G, R, COLP], cdt)
            if mode == "fp16pe":
                ident = pool.tile([128, 128], cdt)
                nc.sync.dma_start(out=ident[:], in_=idn[:])
                acc_ps = pspool.tile([128, NPC * G * HB * W], fp32)
                # touch ACT once so its table set loads during the DMA wait
                warm = pool.tile([128, 1], fp32)
                nc.scalar.copy(out=warm[:], in_=ident[:, 0:1])

            # Zero only the halo regions: left/right column borders of
            # in_pad/in_pad1, and the top/bottom staging row-bands that flow
            # into the padded rows. On the DVE: it is idle at kernel start and
            # clears these fast, unblocking the staging DMAs (WAW).
            # full-partition stage row bands first: they gate the staging DMAs
            # (the DMAs overwrite whichever rows are valid for their block)
            nc.vector.memset(stage[:, :, :, 0:PAD, :], 0.0)
            nc.vector.memset(stage[:, :, :, R - PAD : R, :], 0.0)
            nc.vector.memset(in_pad[:, :, :, :, 0:PAD], 0.0)
            nc.vector.memset(in_pad[:, :, :, :, PAD + W : COLP], 0.0)
            if mode != "fp32":
                # in_pad1 holds input shifted one column left: interior
                # cols [PAD-1, PAD-1+W), borders outside that
                nc.vector.memset(in_pad1[:, :, :, :, 0 : PAD - 1], 0.0)
                nc.vector.memset(in_pad1[:, :, :, :, PAD - 1 + W : COLP], 0.0)

            # Input staging loads on HWDGE (fp32), then two independent
            # convert-copies of stage: ACT builds in_pad (interior at col PAD),
            # DVE builds in_pad1 (interior at col PAD-1, i.e. shifted one
            # column left, which keeps odd-kw taps 4B-aligned for the DVE 2x
            # perf mode). Weights stream via SWDGE dtype-cast DMAs in two
            # kh-chunks so compute can start after chunk A lands.
            stage_dmas = []
            # two row-slabs per (n, blk): slab 1 covers padded rows [0, 8)
            # (everything the kh=0 taps read), slab 2 the rest, so compute can
            # start as soon as slab 1 lands
            SLABS = [(0, HB), (HB, R)] if mode != "fp32" else [(0, R)]
            for ra, rb in SLABS:
                for n in range(NPC):
                    for blk in range(BLK):
                        # valid input rows within padded window rows [ra, rb)
                        h0 = max(0, blk * HB - PAD + ra)
                        h1 = min(H, blk * HB - PAD + rb)
                        if h1 <= h0:
                            continue
                        r0 = h0 - (blk * HB - PAD)
                        dst = stage[blk::BLK, n].rearrange("p g r c -> p g (r c)")[
                            :, :, r0 * W : (r0 + (h1 - h0)) * W
                        ]
                        src = x[n].rearrange("(g cc) h w -> cc g (h w)", g=G)[
                            :, :, h0 * W : h1 * W
                        ]
                        ieng = nc.sync if blk % 2 == 0 else nc.scalar
                        sd = ieng.dma_start(out=dst, in_=src)
                        stage_dmas.append(sd)
            # Convert-copies split across ACT and DVE (DVE single-src fp32
            # copies run at 2x): ACT pads image 0; DVE shifts image 0, then
            # pads+shifts image 1.
            if mode == "fp32":
                for n in range(NPC):
                    nc.scalar.copy(
                        out=in_pad[:, n, :, :, PAD : PAD + W], in_=stage[:, n]
                    )
            else:
                # slab 1 (rows 0:8, the kh=0 critical path): one copy on ACT,
                # three on DVE (2x single-src fp32 copies). Slab 2 goes
                # entirely to ACT, which is idle while kh=0 computes.
                ra, rb = SLABS[0]
                nc.scalar.copy(
                    out=in_pad[:, 0, :, ra:rb, PAD : PAD + W],
                    in_=stage[:, 0, :, ra:rb],
                )
                nc.vector.tensor_copy(
                    out=in_pad1[:, 0, :, ra:rb, PAD - 1 : PAD - 1 + W],
                    in_=stage[:, 0, :, ra:rb],
                )
                nc.vector.tensor_copy(
                    out=in_pad[:, 1, :, ra:rb, PAD : PAD + W],
                    in_=stage[:, 1, :, ra:rb],
                )
                nc.vector.tensor_copy(
                    out=in_pad1[:, 1, :, ra:rb, PAD - 1 : PAD - 1 + W],
                    in_=stage[:, 1, :, ra:rb],
                )
                ra, rb = SLABS[1]
                for n in range(NPC):
                    nc.scalar.copy(
                        out=in_pad[:, n, :, ra:rb, PAD : PAD + W],
                        in_=stage[:, n, :, ra:rb],
                    )
                    nc.scalar.copy(
                        out=in_pad1[:, n, :, ra:rb, PAD - 1 : PAD - 1 + W],
                        in_=stage[:, n, :, ra:rb],
                    )
            # Weight chunks: A loads immediately; later chunks are held back
            # (dep on an early multiply) so their HBM traffic does not starve
            # the input staging path at startup. Deps are attached after the
            # compute loop records its mult instructions.
            KH_CHUNKS = [(0, 1), (1, 2), (2, 4), (4, K)] if mode != "fp32" else [(0, K)]
            CHUNK_DEP_KK = [None, 0, 2, 14]
            wdmas = []  # (chunk_idx, BassInstruction)
            for ci, (kh0, kh1) in enumerate(KH_CHUNKS):
                for n in range(NPC):
                    wsrc = wgt[n].rearrange(
                        "cc kk (blk hb) w -> cc blk kk (hb w)", blk=BLK
                    )
                    for blk in range(BLK):
                        d = nc.gpsimd.dma_start(
                            out=w_t[blk::BLK, n, kh0 * K : kh1 * K].rearrange(
                                "p kk hb w -> p kk (hb w)"
                            ),
                            in_=wsrc[:, blk, kh0 * K : kh1 * K],
                        )
                        wdmas.append((ci, d))

            if mode == "fp32":
                for n in range(NPC):
                    for kh in range(K):
                        for kw in range(K):
                            kk = kh * K + kw
                            in0 = in_pad[:, n, :, kh : kh + HB, kw : kw + W]
                            in1 = (
                                w_t[:, n, kk].unsqueeze(1).broadcast_to([128, G, HB, W])
                            )
                            if kk == 0:
                                nc.vector.tensor_tensor(
                                    out=acc[:, n], in0=in0, in1=in1, op=mult
                                )
                            else:
                                prod = ppool.tile([128, G, HB, W], cdt)
                                nc.vector.tensor_tensor(
                                    out=prod[:], in0=in0, in1=in1, op=mult
                                )
                                nc.vector.tensor_tensor(
                                    out=acc[:, n], in0=acc[:, n], in1=prod[:], op=add
                                )
            elif mode == "fp16pe":
                # DVE computes fp16 products at its 2x rate; the otherwise
                # idle Tensor engine accumulates them into an fp32 PSUM
                # accumulator via identity matmuls (start clears, subsequent
                # matmuls accumulate via PSUM has_written bits). Accumulation
                # is therefore exact fp32: only products are rounded to fp16.
                shp = [128, NPC, G, HB, W]
                FLAT = NPC * G * HB * W  # 4096 fp32 = exactly 8 PSUM banks
                NBANK = FLAT // 512
                mults = []
                for kk in range(KK):
                    kh, kw = divmod(kk, K)
                    if kw % 2 == 0:
                        in0 = in_pad[:, :, :, kh : kh + HB, kw : kw + W]
                    else:
                        in0 = in_pad1[:, :, :, kh : kh + HB, kw - 1 : kw - 1 + W]
                    in1 = w_t[:, :, kk].unsqueeze(2).broadcast_to(shp)
                    prod = ppool.tile(shp, cdt)
                    m = nc.vector.tensor_tensor(
                        out=prod[:], in0=in0, in1=in1, op=mult
                    )
                    mults.append(m)
                    pf = prod[:].rearrange("p n g h w -> p (n g h w)")
                    for b in range(NBANK):
                        nc.tensor.matmul(
                            out=acc_ps[:, 512 * b : 512 * (b + 1)],
                            lhsT=ident[:],
                            rhs=pf[:, 512 * b : 512 * (b + 1)],
                            start=(kk == 0),
                            stop=(kk == KK - 1),
                        )
                # evict PSUM -> SBUF (fp32) in quarters, alternating DVE (fast
                # 2x fp32 copy, idle once the multiplies are done) and ACT, so
                # stores can start while later quarters are still copying
                av = acc[:].rearrange("p n g h w -> p (n g h w)")
                q = FLAT // 4
                for i in range(4):
                    eng = nc.vector.tensor_copy if i % 2 == 0 else nc.scalar.copy
                    eng(out=av[:, i * q : (i + 1) * q], in_=acc_ps[:, i * q : (i + 1) * q])
                from concourse.bass import _add_dep_helper

                for ci, d in wdmas:
                    dep_kk = CHUNK_DEP_KK[ci]
                    if dep_kk == "stage":
                        _add_dep_helper(
                            d.ins,
                            stage_dmas[-1].ins,
                            sync=True,
                            reason="weight chunk A transfers after input staging",
                        )
                    elif dep_kk is not None:
                        _add_dep_helper(
                            d.ins,
                            mults[dep_kk].ins,
                            sync=True,
                            reason=f"delay weight chunk {ci} past startup",
                        )
            else:
                # both images processed by each instruction (n on a free axis)
                shp = [128, NPC, G, HB, W]
                prev_row = None
                treeacc = tpool.tile(shp, cdt, tag="treeacc")
                for kh in range(K):
                    rowacc = rpool.tile(shp, cdt)
                    for kw in range(K):
                        kk = kh * K + kw
                        if kw % 2 == 0:
                            in0 = in_pad[:, :, :, kh : kh + HB, kw : kw + W]
                        else:
                            in0 = in_pad1[:, :, :, kh : kh + HB, kw - 1 : kw - 1 + W]
                        in1 = w_t[:, :, kk].unsqueeze(2).broadcast_to(shp)
                        if kw == 0:
                            nc.vector.tensor_tensor(
                                out=rowacc[:], in0=in0, in1=in1, op=mult
                            )
                        else:
                            prod = ppool.tile(shp, cdt)
                            nc.vector.tensor_tensor(
                                out=prod[:], in0=in0, in1=in1, op=mult
                            )
                            nc.vector.tensor_tensor(
                                out=rowacc[:], in0=rowacc[:], in1=prod[:], op=add
                            )
                    # pairwise fp16 combine of row sums (keeps the fp16 2x
                    # rate; only the very last add writes fp32)
                    if kh == K - 1:
                        # split by g-half so the first half's output stores can
                        # overlap the second half's add
                        half = G // 2
                        nc.vector.tensor_tensor(
                            out=acc[:, :, 0:half],
                            in0=treeacc[:, :, 0:half],
                            in1=rowacc[:, :, 0:half],
                            op=add,
                        )
                        nc.vector.tensor_tensor(
                            out=acc[:, :, half:G],
                            in0=treeacc[:, :, half:G],
                            in1=rowacc[:, :, half:G],
                            op=add,
                        )
                    elif kh % 2 == 1:
                        if kh == 1:
                            nc.vector.tensor_tensor(
                                out=treeacc[:], in0=prev_row[:], in1=rowacc[:], op=add
                            )
                        else:
                            pair = tpool.tile(shp, cdt, tag="pair")
                            nc.vector.tensor_tensor(
                                out=pair[:], in0=prev_row[:], in1=rowacc[:], op=add
                            )
                            nc.vector.tensor_tensor(
                                out=treeacc[:], in0=treeacc[:], in1=pair[:], op=add
                            )
                        prev_row = None
                    else:
                        prev_row = rowacc
            for g in range(G):
                for n in range(NPC):
                    dsty = y[n].rearrange(
                        "(g cc) (blk hb) w -> g cc blk (hb w)", g=G, blk=BLK
                    )
                    deng = nc.sync if g % 2 == 0 else nc.scalar
                    deng.dma_start(
                        out=dsty[g],
                        in_=acc[:, n, g].rearrange("p hb w -> p (hb w)"),
                    )

    nc.compile()
    return nc


def _build_v2(nc, x, wgt, y, idn):
    """Three-engine tap stream.

    Free-dim layout (g, n, hb, w) so PSUM bank b holds group g=b. Per tap:
    DVE multiplies groups 0..6 (fp16 2x), GPSIMD multiplies group 7 (software,
    alignment-insensitive so it reads in_pad directly even for odd kw), PE
    accumulates all 8 banks via identity matmuls. kh=0 taps are split by image
    n (even kw first) so compute starts as soon as image 0's rows [0,8) are
    staged and converted. Weights stream as 8 consolidated SWDGE cast DMAs
    whose generation is self-throttled by gpsimd program order. Eviction
    alternates DVE/ACT per bank with per-bank stores chasing.
    """
    import concourse.mybir as mybir
    import concourse.tile as tile

    fp32 = mybir.dt.float32
    cdt = mybir.dt.float16
    mult = mybir.AluOpType.mult
    GD = G - 1  # groups handled by DVE

    with tile.TileContext(nc) as tc:
        with (
            tc.tile_pool(name="main", bufs=1) as pool,
            tc.tile_pool(name="prod", bufs=6) as ppool,
            tc.tile_pool(name="psum", bufs=1, space="PSUM") as pspool,
        ):
            in_pad = pool.tile([128, G, NPC, R, COLP], cdt)
            in_pad1 = pool.tile([128, G, NPC, R, COLP], cdt)
            stage = pool.tile([128, NPC, G, R, W], fp32)
            w_t = pool.tile([128, NPC, KK, HB, W], cdt)
            acc = pool.tile([128, G, NPC, HB, W], fp32)
            ident = pool.tile([128, 128], cdt)
            acc_ps = pspool.tile([128, G * NPC * HB * W], fp32)  # 8 banks

            # --- weight cast DMAs, all generated up-front on gpsimd ---
            # (SWDGE is the only caster). Rows kh0 and kh1 go as their own
            # small chunks so early taps aren't gated on the bulk; gpsimd's
            # g7 multiplies tolerate the generation time via the delayed
            # bank-7 emission below.
            wre = [
                wgt[n].rearrange("cc kk (blk hb) w -> cc blk kk (hb w)", blk=BLK)
                for n in range(NPC)
            ]

            def wchunk(kh0, kh1, kk0=0):
                for n in range(NPC):
                    for b in range(BLK):
                        nc.gpsimd.dma_start(
                            out=w_t[b::BLK, n, kh0 * K + kk0 : kh1 * K].rearrange(
                                "p kk hb w -> p kk (hb w)"
                            ),
                            in_=wre[n][:, b, kh0 * K + kk0 : kh1 * K],
                        )

            def wkk0():
                for n in range(NPC):
                    for b in range(BLK):
                        nc.gpsimd.dma_start(
                            out=w_t[b::BLK, n, 0:1].rearrange("p kk hb w -> p kk (hb w)"),
                            in_=wre[n][:, b, 0:1],
                        )

            wkk0()
            wchunk(0, 1, kk0=1)
            wchunk(1, 4)
            wchunk(4, K)

            # --- halo memsets on DVE (idle until first mult) ---
            nc.vector.memset(stage[:, :, :, 0:PAD, :], 0.0)
            nc.vector.memset(stage[:, :, :, R - PAD : R, :], 0.0)
            nc.vector.memset(in_pad[:, :, :, :, 0:PAD], 0.0)
            nc.vector.memset(in_pad[:, :, :, :, PAD + W : COLP], 0.0)
            nc.vector.memset(in_pad1[:, :, :, :, 0 : PAD - 1], 0.0)
            nc.vector.memset(in_pad1[:, :, :, :, PAD - 1 + W : COLP], 0.0)
            zs = pool.tile([128, NPC * HB * W], cdt)
            nc.vector.memset(zs[:], 0.0)

            # --- input staging: big DMA per (slab, n) + 3 halo DMAs each ---
            # big: padded rows [3,8) then [8,11) are valid for every blk
            # (h = blk*8 + (r-PAD)); halo rows need per-blk clamping.
            nc.sync.dma_start(out=ident[:], in_=idn[:])
            xbig = [
                x[n].rearrange("(g cc) (blk hb) w -> (cc blk) g hb w", g=G, blk=BLK)
                for n in range(NPC)
            ]
            xh = [
                x[n].rearrange("(g cc) h w -> cc g h w", g=G) for n in range(NPC)
            ]
            for n in range(NPC):
                nc.sync.dma_start(
                    out=stage[:, n, :, PAD : PAD + 5, :], in_=xbig[n][:, :, 0:5, :]
                )
            for n in range(NPC):
                for b in (1, 2, 3):  # top halo rows [0,3): h in [b*8-3, b*8)
                    nc.scalar.dma_start(
                        out=stage[b::BLK, n, :, 0:PAD, :],
                        in_=xh[n][:, :, b * HB - PAD : b * HB, :],
                    )
            for n in range(NPC):
                nc.sync.dma_start(
                    out=stage[:, n, :, PAD + 5 : PAD + HB, :],
                    in_=xbig[n][:, :, 5:HB, :],
                )
            for n in range(NPC):
                for b in (0, 1, 2):  # bottom halo rows [11,14): h in [b*8+8, b*8+11)
                    nc.scalar.dma_start(
                        out=stage[b::BLK, n, :, R - PAD : R, :],
                        in_=xh[n][:, :, b * HB + HB : b * HB + HB + PAD, :],
                    )

            # touch ACT once so its table set loads during the DMA wait
            warm = pool.tile([128, 1], fp32)
            nc.scalar.copy(out=warm[:], in_=ident[:, 0:1])

            # --- converts, all on ACT, ordered for earliest compute ---
            # slab1 (rows 0:8) n0 pad, n1 pad, n0 pad1, n1 pad1; then slab2.
            def conv(n, ra, rb, shifted):
                dst = in_pad1 if shifted else in_pad
                c0 = PAD - 1 if shifted else PAD
                nc.scalar.copy(
                    out=dst[:, :, n, ra:rb, c0 : c0 + W], in_=stage[:, n, :, ra:rb]
                )

            conv(0, 0, HB, False)
            conv(1, 0, HB, False)
            conv(0, 0, HB, True)
            conv(1, 0, HB, True)
            conv(0, HB, R, False)
            conv(1, HB, R, False)
            conv(0, HB, R, True)
            conv(1, HB, R, True)

            # --- PSUM init: zero-matmuls open every bank's accumulation
            # group (HW start_tensor_calc is bank-wide) and warm the PE
            # pipeline during the staging wait. All tap matmuls accumulate.
            shp_j = [128, G, NPC, HB, W]
            SPAN = NPC * HB * W  # 512 elems = one PSUM bank
            for g in range(G):
                nc.tensor.matmul(
                    out=acc_ps[:, SPAN * g : SPAN * (g + 1)],
                    lhsT=ident[:],
                    rhs=zs[:],
                    start=True,
                    stop=False,
                )

            def dve_mult(kk, prod_t):
                kh, kw = divmod(kk, K)
                src, c0 = (in_pad, kw) if kw % 2 == 0 else (in_pad1, kw - 1)
                in0 = src[:, :, :, kh : kh + HB, c0 : c0 + W]
                in1 = w_t[:, :, kk].unsqueeze(1).broadcast_to(shp_j)
                return nc.vector.tensor_tensor(
                    out=prod_t[:], in0=in0, in1=in1, op=mult
                )

            def main_matmuls(prod_t, last=False):
                pf = prod_t[:].rearrange("p g n h w -> p (g n h w)")
                for g in range(G):
                    nc.tensor.matmul(
                        out=acc_ps[:, SPAN * g : SPAN * (g + 1)],
                        lhsT=ident[:],
                        rhs=pf[:, SPAN * g : SPAN * (g + 1)],
                        start=False,
                        stop=last,
                    )

            # joint taps; within kh=0 run even kw first so odd taps wait on
            # in_pad1 converts without stalling the stream
            ORDER = [0, 2, 4, 6, 1, 3, 5] + list(range(K, KK))
            for i, kk in enumerate(ORDER):
                prod_t = ppool.tile([128, G, NPC, HB, W], cdt)
                dve_mult(kk, prod_t)
                main_matmuls(prod_t, last=(kk == KK - 1))

            # --- eviction + stores, per bank, alternating engines/queues ---
            yr = y.rearrange("n (g cc) (blk hb) w -> g (cc blk) n (hb w)", g=G, blk=BLK)
            span = NPC * HB * W
            for g in range(G):
                ev = nc.vector.tensor_copy if g % 2 == 0 else nc.scalar.copy
                ev(
                    out=acc[:, g].rearrange("p n h w -> p (n h w)"),
                    in_=acc_ps[:, span * g : span * (g + 1)],
                )
                deng = nc.sync if g % 2 == 0 else nc.scalar
                deng.dma_start(
                    out=yr[g], in_=acc[:, g].rearrange("p n h w -> p n (h w)")
                )


def _get_nc(mode=None):
    mode = mode or MODE
    if mode not in _cache:
        _cache[mode] = _build(mode)
    return _cache[mode]


def kernel(input_, weight, _trace=False, _mode=None):
    from concourse.bass_utils import run_bass_kernel_spmd

    mode = _mode or MODE
    nc = _get_nc(mode)
    input_ = np.ascontiguousarray(input_, dtype=np.float32)
    weight = np.ascontiguousarray(weight, dtype=np.float32)
    eye = np.eye(128, dtype=np.float16)
    in_maps = [
        {
            "input": input_[i * NPC : (i + 1) * NPC],
            "weight": weight[i * NPC : (i + 1) * NPC],
            **({"identity": eye} if mode in ("fp16pe", "v2") else {}),
        }
        for i in range(NCORES)
    ]
    res = run_bass_kernel_spmd(nc, in_maps, list(range(NCORES)), trace=_trace)
    _cache["last_result"] = res
    out = np.concatenate([res.results[i]["output"] for i in range(NCORES)], axis=0)
    return out

